# revision 37
# baseline (speedup 1.0000x reference)
"""Causal self-attention (B=1, T=4096, C=768, H=12, D=64) on 8 TRN2 NeuronCores.

Sharding: 4 head-groups x 2 query-parity sets.
  core c: head group g = c//2 (heads 3g..3g+3), parity qh = c%2
  (query blocks {2j+qh : j in 0..16} of 128 rows each -- parity
  interleaving balances the causal triangle across the pair).
Each core computes qkv projections for its heads (q only for its own
query rows), flash-style attention without max subtraction (scores are
bounded for this problem's scale), and a partial output projection
restricted to its heads' channels. The host sums the 4 head-group
partials per parity, adds b_out, and reassembles the interleaved rows.

All SPMD cores run one identical program; per-core variation enters only
through data (pre-sliced inputs and a small causal tail-mask tensor).

Layout notes:
  - scores are built transposed, ST[k, q] = (kT tile).T @ qT tile with
    the head dim (64) as contraction; softmax denominators come for free
    from a ones-column appended to v in the PV matmul; normalization is
    applied post-PV via a K=1 broadcast matmul from psum row 64.
  - fp32r matmuls throughout (full PE rate at moving dim >= 256).
  - heads 0,1 are packed into 128-partition tiles (base-64 operand
    slices); head 2's k and v share one 128-partition tile. This keeps
    every PSUM->SBUF drain 128 partitions wide (DVE cost is per free
    element regardless of partition count).
  - phase C runs kt in batches of 3 through a [128,3,512] psum tile so
    score matmuls stay ahead of the exp->PV chain instead of
    interleaving with it (in-order PE queue stalls otherwise).
"""

import numpy as np
from contextlib import ExitStack

import concourse.bass as bass  # noqa: F401
import concourse.mybir as mybir
import concourse.tile as tile
from concourse import bacc
from concourse import bass_utils
from concourse.masks import make_identity

T, C, H, D = 4096, 768, 12, 64
N_CORES = 8
HPG = 3
GCH = HPG * D              # 192 channels per group per tensor
TQ = T // 2                # 2048 query rows per core
NTT = T // 128             # 32 key tiles
NQT = TQ // 128            # 16 query tiles per core
NST = TQ // 512            # 4 query supertiles per core
KO = C // 128              # 6 contraction subtiles
PW = 1024                  # transpose panel width

F32 = mybir.dt.float32
F32R = mybir.dt.float32r
AF = mybir.ActivationFunctionType
ALU = mybir.AluOpType

_CACHE = {}
_BIG_EXP = True
_CHUNK_TR = True
_STOP_AFTER = "full"  # "AB" | "C" | "full"


def build_nc():
    nc = bacc.Bacc(
        "TRN2", target_bir_lowering=False, debug=False, num_devices=N_CORES
    )

    x = nc.dram_tensor("x", [T, C], F32R, kind="ExternalInput").ap()
    xq = nc.dram_tensor("xq", [TQ, C], F32R, kind="ExternalInput").ap()
    wq_d = nc.dram_tensor("wq", [C, GCH], F32R, kind="ExternalInput").ap()
    wk_d = nc.dram_tensor("wk", [C, GCH], F32R, kind="ExternalInput").ap()
    wv_d = nc.dram_tensor("wv", [C, GCH], F32R, kind="ExternalInput").ap()
    bq_d = nc.dram_tensor("bq", [GCH], F32R, kind="ExternalInput").ap()
    bk_d = nc.dram_tensor("bk", [GCH], F32R, kind="ExternalInput").ap()
    bv_d = nc.dram_tensor("bv", [GCH], F32R, kind="ExternalInput").ap()
    wo_d = nc.dram_tensor("wo", [GCH, C], F32R, kind="ExternalInput").ap()
    tm_d = nc.dram_tensor("tmask", [128, 8, 128], F32R, kind="ExternalInput").ap()
    out = nc.dram_tensor("out", [TQ, C], F32, kind="ExternalOutput").ap()

    with tile.TileContext(nc) as tc, ExitStack() as ctx:
        wpool = ctx.enter_context(tc.tile_pool(name="weights", bufs=1))
        dpool = ctx.enter_context(tc.tile_pool(name="data", bufs=1))

        # --- weights / constants ---
        wq_sb = wpool.tile([128, KO, GCH], F32R, name="wq_sb")
        wk_sb = wpool.tile([128, KO, GCH], F32R, name="wk_sb")
        wv_sb = wpool.tile([128, KO, GCH], F32R, name="wv_sb")
        for sb, dr in ((wq_sb, wq_d), (wk_sb, wk_d), (wv_sb, wv_d)):
            nc.sync.dma_start(sb[:], dr.rearrange("(ko p) n -> p ko n", p=128))
        # head-2 k (cols 0:64) and head-2 v (cols 64:128) combined
        wkv1_sb = wpool.tile([128, KO, 128], F32R, name="wkv1_sb")
        nc.sync.dma_start(
            wkv1_sb[:, :, 0:64],
            wk_d[:, 128:192].rearrange("(ko p) n -> p ko n", p=128),
        )
        nc.sync.dma_start(
            wkv1_sb[:, :, 64:128],
            wv_d[:, 128:192].rearrange("(ko p) n -> p ko n", p=128),
        )
        wo_sb = [wpool.tile([64, C], F32R, name=f"wo{h}") for h in range(HPG)]
        for h in range(HPG):
            nc.sync.dma_start(wo_sb[h][:], wo_d[h * 64 : (h + 1) * 64, :])

        def bias_tile(name, dr, lo, hi):
            t = wpool.tile([hi - lo, 1], F32R, name=name)
            nc.sync.dma_start(t[:], dr[lo:hi].rearrange("(o p) -> p o", p=hi - lo))
            return t

        bq2 = bias_tile("bq2", bq_d, 0, 128)
        bq1 = bias_tile("bq1", bq_d, 128, 192)
        bk2 = bias_tile("bk2", bk_d, 0, 128)
        bv2 = bias_tile("bv2", bv_d, 0, 128)
        bkv1 = wpool.tile([128, 1], F32R, name="bkv1")
        nc.sync.dma_start(bkv1[0:64, :], bk_d[128:192].rearrange("(o p) -> p o", p=64))
        nc.sync.dma_start(bkv1[64:128, :], bv_d[128:192].rearrange("(o p) -> p o", p=64))

        tm_sb = wpool.tile([128, 8, 128], F32R, name="tm_sb")
        nc.sync.dma_start(tm_sb[:], tm_d[:])
        ident32 = wpool.tile([128, 128], F32, name="ident32")
        make_identity(nc, ident32[:])
        ident = wpool.tile([128, 128], F32R, name="ident")
        nc.vector.tensor_copy(ident[:], ident32[:])
        ones65_32 = wpool.tile([65, 64], F32, name="ones65_32")
        nc.vector.memset(ones65_32[:], 1.0)
        ones65 = wpool.tile([65, 64], F32R, name="ones65")
        nc.vector.tensor_copy(ones65[:], ones65_32[:])
        onescol = wpool.tile([128, NTT], F32, name="onescol")
        nc.vector.memset(onescol[:], 1.0)
        zeros384 = wpool.tile([128, 384], F32, name="zeros384")
        nc.vector.memset(zeros384[:], 0.0)

        # --- persistent tensors ---
        qT2 = dpool.tile([128, TQ], F32R, name="qT2")     # q heads 0,1
        qT1 = dpool.tile([64, TQ], F32R, name="qT1")      # q head 2
        kT2 = dpool.tile([128, T], F32R, name="kT2")      # k heads 0,1
        kvT1 = dpool.tile([128, T], F32R, name="kvT1")    # k head 2 / v head 2
        vaug = [dpool.tile([128, NTT, 72], F32R, name=f"v{h}") for h in range(HPG)]
        attnT = [dpool.tile([64, TQ], F32R, name=f"aT{h}") for h in range(HPG)]
        for h in range(HPG):
            nc.vector.tensor_copy(vaug[h][:, :, 64], onescol[:])

        def s_lhsT(h, ksl):  # kT slice for head h over key slice ksl
            if h == 0:
                return kT2[0:64, ksl]
            if h == 1:
                return kT2[64:128, ksl]
            return kvT1[0:64, ksl]

        def s_rhs(h, qsl):
            if h == 0:
                return qT2[0:64, qsl]
            if h == 1:
                return qT2[64:128, qsl]
            return qT1[0:64, qsl]

        # --- phase A/B ---
        with (
            tc.tile_pool(name="panel", bufs=2) as panpool,
            tc.tile_pool(name="stage", bufs=5) as stpool,
            tc.tile_pool(name="vt", bufs=1) as vtpool,
            tc.tile_pool(name="ab_ps", bufs=2, space="PSUM") as abps,
            tc.tile_pool(name="ab1_ps", bufs=1, space="PSUM") as abps1,
        ):

            def do_panel(src_ap, row0, panelT):
                """Transpose PW rows of src into panelT [128, KO, PW]."""
                if not _CHUNK_TR:
                    for tt in range(PW // 128):
                        st_t = stpool.tile([128, C], F32R, tag="stage")
                        r = row0 + tt * 128
                        nc.sync.dma_start(st_t[:], src_ap[r : r + 128, :])
                        for cc in range(KO):
                            ps = abps.tile([128, 128], F32R, tag="tr")
                            nc.tensor.transpose(
                                ps[:], st_t[:, cc * 128 : (cc + 1) * 128], ident[:]
                            )
                            nc.vector.tensor_copy(
                                panelT[:, cc, tt * 128 : (tt + 1) * 128], ps[:]
                            )
                    return
                for grp in range(PW // 512):
                    stages = []
                    for j in range(4):
                        st_t = stpool.tile([128, C], F32R, tag="stage")
                        r = row0 + grp * 512 + j * 128
                        nc.sync.dma_start(st_t[:], src_ap[r : r + 128, :])
                        stages.append(st_t)
                    for cc in range(KO):
                        ps = abps.tile([128, 512], F32, tag="tr")
                        for j in range(4):
                            # x_tile.T via a NORMAL matmul against identity:
                            # transpose-mode doesn't count as PE-busy for the
                            # HAM clock gate; this does, keeping the PE warm.
                            nc.tensor.matmul(
                                ps[:, j * 128 : (j + 1) * 128],
                                stages[j][:, cc * 128 : (cc + 1) * 128],
                                ident[:],
                                start=True,
                                stop=True,
                            )
                        nc.vector.tensor_copy(
                            panelT[:, cc, grp * 512 : (grp + 1) * 512], ps[:]
                        )

            def proj(panelT, w_sb, csl, bias, dest, off, m):
                """dest[:, off:...] = w_sb[:, :, csl].T @ panelT + bias."""
                for st in range(PW // 512):
                    tag = "proj" if m == 128 else "proj1"
                    pool_ = abps if m == 128 else abps1
                    ps = pool_.tile([m, 512], F32, tag=tag)
                    for ko in range(KO):
                        nc.tensor.matmul(
                            ps[:],
                            w_sb[:, ko, csl],
                            panelT[:, ko, st * 512 : (st + 1) * 512],
                            start=(ko == 0),
                            stop=(ko == KO - 1),
                        )
                    nc.vector.tensor_tensor(
                        dest[:, off + st * 512 : off + (st + 1) * 512],
                        ps[:],
                        bias[:].to_broadcast([m, 512]),
                        ALU.add,
                    )

            def emit_projs(pan, kind, p):
                if kind == "q":
                    proj(pan, wq_sb, slice(0, 128), bq2, qT2, p * PW, 128)
                    proj(pan, wq_sb, slice(128, 192), bq1, qT1, p * PW, 64)
                    return
                proj(pan, wk_sb, slice(0, 128), bk2, kT2, p * PW, 128)
                proj(pan, wkv1_sb, slice(0, 128), bkv1, kvT1, p * PW, 128)
                vT2 = vtpool.tile([128, PW], F32R, tag="vT2", name="vT2")
                proj(pan, wv_sb, slice(0, 128), bv2, vT2, 0, 128)
                # transpose v tiles into [t, d] layout (+ ones column)
                for tt in range(PW // 128):
                    gt = p * (PW // 128) + tt
                    tsl = slice(tt * 128, (tt + 1) * 128)
                    gsl = slice(p * PW + tt * 128, p * PW + (tt + 1) * 128)
                    for h, (src, ssl, isl) in enumerate(
                        (
                            (vT2, slice(0, 64), slice(0, 64)),
                            (vT2, slice(64, 128), slice(64, 128)),
                            (kvT1, slice(64, 128), slice(64, 128)),
                        )
                    ):
                        ps = abps.tile([128, 64], F32, tag="vtr")
                        insl = tsl if h < 2 else gsl
                        nc.tensor.matmul(
                            ps[:], src[ssl, insl], ident[isl, isl][:, 0:64],
                            start=True, stop=True,
                        )
                        nc.vector.tensor_copy(vaug[h][:, gt, 0:64], ps[:])

            # software-pipelined: panel p+1's transposes are emitted before
            # panel p's projections so the PE never waits on the DVE
            # psum->panel copies (contiguous PE work keeps the HAM warm).
            panels = [("q", p) for p in range(TQ // PW)] + [
                ("kv", p) for p in range(T // PW)
            ]
            prev = None
            for kind, p in panels:
                pan = panpool.tile([128, KO, PW], F32R, tag="panel")
                do_panel(xq if kind == "q" else x, p * PW, pan)
                if prev is not None:
                    emit_projs(*prev)
                prev = (pan, kind, p)
            emit_projs(*prev)

        # --- phase C: attention ---
        # Software-pipelined: score batches run two batches ahead of the
        # exp-gated PV matmuls, and each unit's normalization is emitted
        # inside the next unit's stream, so the PE instruction queue never
        # parks behind a ScalarE/VectorE dependency (contiguous PE work is
        # required to get and keep the HAM clock at 2.4 GHz).
        BK = 2  # kt batch
        LAG = 2  # batches between S and PV
        with (
            tc.tile_pool(name="pe", bufs=2 + LAG) as pepool,
            tc.tile_pool(name="rc", bufs=2) as rcpool,
            tc.tile_pool(name="s_ps", bufs=2, space="PSUM") as sps,
            tc.tile_pool(name="a_ps", bufs=2, space="PSUM") as apsp,
            tc.tile_pool(name="r_ps", bufs=1, space="PSUM") as rps,
        ):
            units = [
                (h, s)
                for h in range(HPG if _STOP_AFTER != "AB" else 0)
                for s in range(NST)
            ]

            def start_norm(h, s, a_ps):
                # reciprocal of a [1,512] row runs on one DVE lane (~3.3us);
                # kick it off now, finish the broadcast+scale much later so
                # the PE-side rep matmul never waits on it.
                rc = rcpool.tile([65, 512], F32R, tag="rc")
                with nc.allow_low_precision("f32r is wire-identical to f32"):
                    nc.vector.reciprocal(rc[64:65, :], a_ps[64:65, :])
                return (h, s, a_ps, rc)

            def finish_norm(h, s, a_ps, rc):
                qsl = slice(s * 512, (s + 1) * 512)
                r_ps = rps.tile([64, 512], F32, tag="rep")
                nc.tensor.matmul(
                    r_ps[:], ones65[64:65, :], rc[64:65, :], start=True, stop=True
                )
                an = rcpool.tile([64, 512], F32, tag="an")
                nc.vector.tensor_copy(an[:], a_ps[0:64, :])
                nc.vector.tensor_tensor(attnT[h][:, qsl], an[:], r_ps[:], ALU.mult)

            def emit_exp(h, s, kts, bs, pe_t):
                if kts[-1] < 8 * s and _BIG_EXP:
                    nc.scalar.activation(
                        pe_t[:, 0 : len(kts), :],
                        bs[:, 0 : len(kts), :],
                        AF.Exp,
                        scale=0.125,
                    )
                    return
                for j, kt in enumerate(kts):
                    if kt < 8 * s:
                        nc.scalar.activation(
                            pe_t[:, j, :], bs[:, j, :], AF.Exp, scale=0.125
                        )
                        continue
                    ktp = kt - 8 * s
                    cgm = ktp // 2
                    if cgm > 0:
                        nc.vector.tensor_copy(
                            pe_t[:, j, 0 : cgm * 128], zeros384[:, 0 : cgm * 128]
                        )
                    nc.scalar.activation(
                        pe_t[:, j, cgm * 128 :],
                        bs[:, j, cgm * 128 :],
                        AF.Exp,
                        scale=0.125,
                    )
                    nc.vector.tensor_tensor(
                        pe_t[:, j, cgm * 128 : (cgm + 1) * 128],
                        pe_t[:, j, cgm * 128 : (cgm + 1) * 128],
                        tm_sb[:, ktp, :],
                        ALU.mult,
                    )

            # pipeline state
            pend_pv = []    # (h, s, a_ps, pe_t, kts, nkt)
            pend_norm = []  # (due_batch, norm_args)
            batch_no = [0]

            def flush_pv(keep):
                while len(pend_pv) > keep:
                    h, s, a_ps, pe_t, kts, nkt = pend_pv.pop(0)
                    for j, kt in enumerate(kts):
                        nc.tensor.matmul(
                            a_ps[:],
                            vaug[h][:, kt, 0:65],
                            pe_t[:, j, :],
                            start=(kt == 0),
                            stop=(kt == nkt - 1),
                        )
                    if kts[-1] == nkt - 1:
                        pend_norm.append((batch_no[0] + 4, start_norm(h, s, a_ps)))

            def flush_norms(force=False):
                while pend_norm and (force or pend_norm[0][0] <= batch_no[0]):
                    _, args = pend_norm.pop(0)
                    finish_norm(*args)

            for h, s in units:
                nkt = 8 * s + 8
                # backstop: a_ps slots recycle every 2 units, so any norm
                # still pending must be emitted before this unit's alloc
                flush_norms(force=True)
                a_ps = apsp.tile([65, 512], F32, tag="attn")
                qsl = slice(s * 512, (s + 1) * 512)
                for kt0 in range(0, nkt, BK):
                    kts = list(range(kt0, min(kt0 + BK, nkt)))
                    bs = sps.tile([128, BK, 512], F32, tag="s")
                    for j, kt in enumerate(kts):
                        nc.tensor.matmul(
                            bs[:, j, :],
                            s_lhsT(h, slice(kt * 128, (kt + 1) * 128)),
                            s_rhs(h, qsl),
                            start=True,
                            stop=True,
                        )
                    batch_no[0] += 1
                    flush_pv(LAG)
                    flush_norms()
                    pe_t = pepool.tile([128, BK, 512], F32R, tag="pe")
                    emit_exp(h, s, kts, bs, pe_t)
                    pend_pv.append((h, s, a_ps, pe_t, kts, nkt))
            flush_pv(0)
            flush_norms(force=True)

        # --- phase D: partial output projection ---
        with (
            tc.tile_pool(name="ob", bufs=3) as opool,
            tc.tile_pool(name="d_ps", bufs=2, space="PSUM") as dps,
        ):
            for tj in range(NQT if _STOP_AFTER == "full" else 0):
                tsl = slice(tj * 128, (tj + 1) * 128)
                p1 = dps.tile([128, 512], F32, tag="o1")
                p2 = dps.tile([128, 256], F32, tag="o2")
                lhs = tuple(attnT[h][:, tsl] for h in range(HPG))
                rhs2 = tuple(wo_sb[h][:, :] for h in range(HPG))
                for h in range(HPG):
                    nc.tensor.matmul(
                        p1[:], lhs[h], rhs2[h][:, 0:512],
                        start=(h == 0), stop=(h == HPG - 1),
                    )
                for h in range(HPG):
                    nc.tensor.matmul(
                        p2[:], lhs[h], rhs2[h][:, 512:768],
                        start=(h == 0), stop=(h == HPG - 1),
                    )
                ob = opool.tile([128, C], F32, tag="ob")
                nc.vector.tensor_copy(ob[:, 0:512], p1[:])
                nc.vector.tensor_copy(ob[:, 512:768], p2[:])
                nc.sync.dma_start(out[tsl, :], ob[:])

    nc.compile()
    return nc


def _get_nc():
    if "nc" not in _CACHE:
        _CACHE["nc"] = build_nc()
    return _CACHE["nc"]


def make_in_maps(inputs):
    """Shard full inputs into 8 per-core input maps."""
    x = np.ascontiguousarray(np.asarray(inputs["x"], dtype=np.float32)).reshape(T, C)
    W_qkv = np.asarray(inputs["W_qkv"], dtype=np.float32)
    b_qkv = np.asarray(inputs["b_qkv"], dtype=np.float32)
    W_out = np.asarray(inputs["W_out"], dtype=np.float32)

    diag = (np.arange(128)[None, :] >= np.arange(128)[:, None]).astype(np.float32)
    ones = np.ones((128, 128), np.float32)
    zeros = np.zeros((128, 128), np.float32)
    tmask = {}
    for qh in (0, 1):
        m = np.empty((128, 8, 128), np.float32)
        for ktp in range(8):
            if qh == 0:
                m[:, ktp] = diag if ktp % 2 == 0 else zeros
            else:
                m[:, ktp] = ones if ktp % 2 == 0 else diag
        tmask[qh] = m

    xr = x.reshape(NTT, 128, C)
    in_maps = []
    for c in range(N_CORES):
        g, qh = c // 2, c % 2
        sl = slice(g * GCH, (g + 1) * GCH)
        in_maps.append(
            {
                "x": x,
                "xq": np.ascontiguousarray(xr[qh::2].reshape(TQ, C)),
                "wq": np.ascontiguousarray(W_qkv[:, 0 * C + g * GCH : 0 * C + (g + 1) * GCH]),
                "wk": np.ascontiguousarray(W_qkv[:, 1 * C + g * GCH : 1 * C + (g + 1) * GCH]),
                "wv": np.ascontiguousarray(W_qkv[:, 2 * C + g * GCH : 2 * C + (g + 1) * GCH]),
                "bq": np.ascontiguousarray(b_qkv[0 * C + g * GCH : 0 * C + (g + 1) * GCH]),
                "bk": np.ascontiguousarray(b_qkv[1 * C + g * GCH : 1 * C + (g + 1) * GCH]),
                "bv": np.ascontiguousarray(b_qkv[2 * C + g * GCH : 2 * C + (g + 1) * GCH]),
                "wo": np.ascontiguousarray(W_out[sl, :]),
                "tmask": tmask[qh],
            }
        )
    return in_maps


def combine_outputs(parts, b_out):
    """Sum head-group partials per parity, reassemble rows, add bias."""
    out = np.zeros((T, C), np.float32)
    orow = out.reshape(NTT, 128, C)
    for qh in (0, 1):
        acc = parts[qh].astype(np.float32).copy()
        for g in range(1, 4):
            acc += parts[2 * g + qh]
        orow[qh::2] = acc.reshape(NQT, 128, C)
    out += np.asarray(b_out, dtype=np.float32)[None, :]
    return out.reshape(1, T, C)


def _run(inputs, trace=False, tmpdir=None):
    nc = _get_nc()
    in_maps = make_in_maps(inputs)
    res = bass_utils.run_bass_kernel_spmd(
        nc, in_maps, core_ids=list(range(N_CORES)), trace=trace, tmpdir=tmpdir
    )
    parts = [np.asarray(res.results[c]["out"]) for c in range(N_CORES)]
    return combine_outputs(parts, inputs["b_out"]), res


def kernel(**inputs):
    out, _ = _run(inputs)
    return out


# revision 39
# speedup vs baseline: 1.2663x; 1.2663x over previous
"""Causal self-attention (B=1, T=4096, C=768, H=12, D=64) on 8 TRN2 NeuronCores.

Sharding: 4 head-groups x 2 query-parity sets.
  core c: head group g = c//2 (heads 3g..3g+3), parity qh = c%2
  (query blocks {2j+qh : j in 0..16} of 128 rows each -- parity
  interleaving balances the causal triangle across the pair).
Each core computes qkv projections for its heads (q only for its own
query rows), flash-style attention without max subtraction (scores are
bounded for this problem's scale), and a partial output projection
restricted to its heads' channels. The host sums the 4 head-group
partials per parity, adds b_out, and reassembles the interleaved rows.

All SPMD cores run one identical program; per-core variation enters only
through data (pre-sliced inputs and a small causal tail-mask tensor).

Layout notes:
  - scores are built transposed, ST[k, q] = (kT tile).T @ qT tile with
    the head dim (64) as contraction; softmax denominators come for free
    from a ones-column appended to v in the PV matmul; normalization is
    applied post-PV via a K=1 broadcast matmul from psum row 64.
  - fp32r matmuls throughout (full PE rate at moving dim >= 256).
  - heads 0,1 are packed into 128-partition tiles (base-64 operand
    slices); head 2's k and v share one 128-partition tile. This keeps
    every PSUM->SBUF drain 128 partitions wide (DVE cost is per free
    element regardless of partition count).
  - phase C runs kt in batches of 3 through a [128,3,512] psum tile so
    score matmuls stay ahead of the exp->PV chain instead of
    interleaving with it (in-order PE queue stalls otherwise).
"""

import numpy as np
from contextlib import ExitStack

import concourse.bass as bass  # noqa: F401
import concourse.mybir as mybir
import concourse.tile as tile
from concourse import bacc
from concourse import bass_utils
from concourse.masks import make_identity

T, C, H, D = 4096, 768, 12, 64
N_CORES = 8
HPG = 3
GCH = HPG * D              # 192 channels per group per tensor
TQ = T // 2                # 2048 query rows per core
NTT = T // 128             # 32 key tiles
NQT = TQ // 128            # 16 query tiles per core
NST = TQ // 512            # 4 query supertiles per core
KO = C // 128              # 6 contraction subtiles
PW = 1024                  # transpose panel width

F32 = mybir.dt.float32
F32R = mybir.dt.float32r
AF = mybir.ActivationFunctionType
ALU = mybir.AluOpType

_CACHE = {}
_BIG_EXP = True
_CHUNK_TR = True
_STOP_AFTER = "full"  # "AB" | "C" | "full"


def build_nc():
    nc = bacc.Bacc(
        "TRN2", target_bir_lowering=False, debug=False, num_devices=N_CORES
    )

    x = nc.dram_tensor("x", [T, C], F32R, kind="ExternalInput").ap()
    xq = nc.dram_tensor("xq", [TQ, C], F32R, kind="ExternalInput").ap()
    wq_d = nc.dram_tensor("wq", [C, GCH], F32R, kind="ExternalInput").ap()
    wk_d = nc.dram_tensor("wk", [C, GCH], F32R, kind="ExternalInput").ap()
    wv_d = nc.dram_tensor("wv", [C, GCH], F32R, kind="ExternalInput").ap()
    bq_d = nc.dram_tensor("bq", [GCH], F32R, kind="ExternalInput").ap()
    bk_d = nc.dram_tensor("bk", [GCH], F32R, kind="ExternalInput").ap()
    bv_d = nc.dram_tensor("bv", [GCH], F32R, kind="ExternalInput").ap()
    wo_d = nc.dram_tensor("wo", [GCH, C], F32R, kind="ExternalInput").ap()
    tm_d = nc.dram_tensor("tmask", [128, 8, 128], F32R, kind="ExternalInput").ap()
    out = nc.dram_tensor("out", [TQ, C], F32, kind="ExternalOutput").ap()

    with tile.TileContext(nc) as tc, ExitStack() as ctx:
        wpool = ctx.enter_context(tc.tile_pool(name="weights", bufs=1))
        dpool = ctx.enter_context(tc.tile_pool(name="data", bufs=1))

        # --- weights / constants ---
        wq_sb = wpool.tile([128, KO, GCH], F32R, name="wq_sb")
        wk_sb = wpool.tile([128, KO, GCH], F32R, name="wk_sb")
        wv_sb = wpool.tile([128, KO, GCH], F32R, name="wv_sb")
        for sb, dr in ((wq_sb, wq_d), (wk_sb, wk_d), (wv_sb, wv_d)):
            nc.sync.dma_start(sb[:], dr.rearrange("(ko p) n -> p ko n", p=128))
        # head-2 k (cols 0:64) and head-2 v (cols 64:128) combined
        wkv1_sb = wpool.tile([128, KO, 128], F32R, name="wkv1_sb")
        nc.sync.dma_start(
            wkv1_sb[:, :, 0:64],
            wk_d[:, 128:192].rearrange("(ko p) n -> p ko n", p=128),
        )
        nc.sync.dma_start(
            wkv1_sb[:, :, 64:128],
            wv_d[:, 128:192].rearrange("(ko p) n -> p ko n", p=128),
        )
        wo_sb = [wpool.tile([64, C], F32R, name=f"wo{h}") for h in range(HPG)]
        for h in range(HPG):
            nc.sync.dma_start(wo_sb[h][:], wo_d[h * 64 : (h + 1) * 64, :])

        def bias_tile(name, dr, lo, hi):
            t = wpool.tile([hi - lo, 1], F32R, name=name)
            nc.sync.dma_start(t[:], dr[lo:hi].rearrange("(o p) -> p o", p=hi - lo))
            return t

        bq2 = bias_tile("bq2", bq_d, 0, 128)
        bq1 = bias_tile("bq1", bq_d, 128, 192)
        bk2 = bias_tile("bk2", bk_d, 0, 128)
        bv2 = bias_tile("bv2", bv_d, 0, 128)
        bkv1 = wpool.tile([128, 1], F32R, name="bkv1")
        nc.sync.dma_start(bkv1[0:64, :], bk_d[128:192].rearrange("(o p) -> p o", p=64))
        nc.sync.dma_start(bkv1[64:128, :], bv_d[128:192].rearrange("(o p) -> p o", p=64))

        tm_sb = wpool.tile([128, 8, 128], F32R, name="tm_sb")
        nc.sync.dma_start(tm_sb[:], tm_d[:])
        ident32 = wpool.tile([128, 128], F32, name="ident32")
        make_identity(nc, ident32[:])
        ident = wpool.tile([128, 128], F32R, name="ident")
        nc.vector.tensor_copy(ident[:], ident32[:])
        ones65_32 = wpool.tile([65, 64], F32, name="ones65_32")
        nc.vector.memset(ones65_32[:], 1.0)
        ones65 = wpool.tile([65, 64], F32R, name="ones65")
        nc.vector.tensor_copy(ones65[:], ones65_32[:])
        onescol = wpool.tile([128, NTT], F32, name="onescol")
        nc.vector.memset(onescol[:], 1.0)
        zeros384 = wpool.tile([128, 384], F32, name="zeros384")
        nc.vector.memset(zeros384[:], 0.0)

        # --- persistent tensors ---
        qT2 = dpool.tile([128, TQ], F32R, name="qT2")     # q heads 0,1
        qT1 = dpool.tile([64, TQ], F32R, name="qT1")      # q head 2
        kT2 = dpool.tile([128, T], F32R, name="kT2")      # k heads 0,1
        kvT1 = dpool.tile([128, T], F32R, name="kvT1")    # k head 2 / v head 2
        vaug = [dpool.tile([128, NTT, 72], F32R, name=f"v{h}") for h in range(HPG)]
        attnT = [dpool.tile([64, TQ], F32R, name=f"aT{h}") for h in range(HPG)]
        for h in range(HPG):
            nc.vector.tensor_copy(vaug[h][:, :, 64], onescol[:])

        def s_lhsT(h, ksl):  # kT slice for head h over key slice ksl
            if h == 0:
                return kT2[0:64, ksl]
            if h == 1:
                return kT2[64:128, ksl]
            return kvT1[0:64, ksl]

        def s_rhs(h, qsl):
            if h == 0:
                return qT2[0:64, qsl]
            if h == 1:
                return qT2[64:128, qsl]
            return qT1[0:64, qsl]

        # --- phase A/B ---
        with (
            tc.tile_pool(name="panel", bufs=2) as panpool,
            tc.tile_pool(name="stage", bufs=2) as stpool,
            tc.tile_pool(name="vt", bufs=1) as vtpool,
            tc.tile_pool(name="ab_ps", bufs=2, space="PSUM") as abps,
            tc.tile_pool(name="ab1_ps", bufs=1, space="PSUM") as abps1,
        ):

            def do_panel(src_ap, row0, panelT):
                """Transpose PW rows of src into panelT [128, KO, PW]."""
                if not _CHUNK_TR:
                    for tt in range(PW // 128):
                        st_t = stpool.tile([128, C], F32R, tag="stage")
                        r = row0 + tt * 128
                        nc.sync.dma_start(st_t[:], src_ap[r : r + 128, :])
                        for cc in range(KO):
                            ps = abps.tile([128, 128], F32R, tag="tr")
                            nc.tensor.transpose(
                                ps[:], st_t[:, cc * 128 : (cc + 1) * 128], ident[:]
                            )
                            nc.vector.tensor_copy(
                                panelT[:, cc, tt * 128 : (tt + 1) * 128], ps[:]
                            )
                    return
                for grp in range(PW // 512):
                    st4 = stpool.tile([128, 4, C], F32R, tag="stage")
                    r = row0 + grp * 512
                    nc.sync.dma_start(
                        st4[:], src_ap[r : r + 512, :].rearrange("(j p) c -> p j c", p=128)
                    )
                    stages = [st4[:, j] for j in range(4)]
                    for cc in range(KO):
                        ps = abps.tile([128, 512], F32R, tag="tr")
                        for j in range(4):
                            nc.tensor.transpose(
                                ps[:, j * 128 : (j + 1) * 128],
                                stages[j][:, cc * 128 : (cc + 1) * 128],
                                ident[:],
                            )
                        nc.vector.tensor_copy(
                            panelT[:, cc, grp * 512 : (grp + 1) * 512], ps[:]
                        )

            def proj(panelT, w_sb, csl, bias, dest, off, m):
                """dest[:, off:...] = w_sb[:, :, csl].T @ panelT + bias."""
                for st in range(PW // 512):
                    tag = "proj" if m == 128 else "proj1"
                    pool_ = abps if m == 128 else abps1
                    ps = pool_.tile([m, 512], F32, tag=tag)
                    for ko in range(KO):
                        nc.tensor.matmul(
                            ps[:],
                            w_sb[:, ko, csl],
                            panelT[:, ko, st * 512 : (st + 1) * 512],
                            start=(ko == 0),
                            stop=(ko == KO - 1),
                        )
                    nc.vector.tensor_tensor(
                        dest[:, off + st * 512 : off + (st + 1) * 512],
                        ps[:],
                        bias[:].to_broadcast([m, 512]),
                        ALU.add,
                    )

            def emit_projs(pan, kind, p):
                if kind == "q":
                    proj(pan, wq_sb, slice(0, 128), bq2, qT2, p * PW, 128)
                    proj(pan, wq_sb, slice(128, 192), bq1, qT1, p * PW, 64)
                    return
                proj(pan, wk_sb, slice(0, 128), bk2, kT2, p * PW, 128)
                proj(pan, wkv1_sb, slice(0, 128), bkv1, kvT1, p * PW, 128)
                vT2 = vtpool.tile([128, PW], F32R, tag="vT2", name="vT2")
                proj(pan, wv_sb, slice(0, 128), bv2, vT2, 0, 128)
                # transpose v tiles into [t, d] layout (+ ones column)
                for tt in range(PW // 128):
                    gt = p * (PW // 128) + tt
                    tsl = slice(tt * 128, (tt + 1) * 128)
                    gsl = slice(p * PW + tt * 128, p * PW + (tt + 1) * 128)
                    for h, (src, ssl, isl) in enumerate(
                        (
                            (vT2, slice(0, 64), slice(0, 64)),
                            (vT2, slice(64, 128), slice(64, 128)),
                            (kvT1, slice(64, 128), slice(64, 128)),
                        )
                    ):
                        ps = abps.tile([128, 64], F32R, tag="vtr")
                        insl = tsl if h < 2 else gsl
                        nc.tensor.transpose(
                            ps[:], src[ssl, insl], ident[isl, isl]
                        )
                        nc.vector.tensor_copy(vaug[h][:, gt, 0:64], ps[:])

            # software-pipelined: panel p+1's transposes are emitted before
            # panel p's projections so the PE never waits on the DVE
            # psum->panel copies (contiguous PE work keeps the HAM warm).
            panels = [("q", p) for p in range(TQ // PW)] + [
                ("kv", p) for p in range(T // PW)
            ]
            prev = None
            for kind, p in panels:
                pan = panpool.tile([128, KO, PW], F32R, tag="panel")
                do_panel(xq if kind == "q" else x, p * PW, pan)
                if prev is not None:
                    emit_projs(*prev)
                prev = (pan, kind, p)
            emit_projs(*prev)

        # --- phase C: attention ---
        # Software-pipelined: score batches run two batches ahead of the
        # exp-gated PV matmuls, and each unit's normalization is emitted
        # inside the next unit's stream, so the PE instruction queue never
        # parks behind a ScalarE/VectorE dependency (contiguous PE work is
        # required to get and keep the HAM clock at 2.4 GHz).
        BK = 2  # kt batch
        LAG = 2  # batches between S and PV
        with (
            tc.tile_pool(name="pe", bufs=2 + LAG) as pepool,
            tc.tile_pool(name="rc", bufs=2) as rcpool,
            tc.tile_pool(name="s_ps", bufs=2, space="PSUM") as sps,
            tc.tile_pool(name="a_ps", bufs=2, space="PSUM") as apsp,
            tc.tile_pool(name="r_ps", bufs=1, space="PSUM") as rps,
        ):
            units = [
                (h, s)
                for h in range(HPG if _STOP_AFTER != "AB" else 0)
                for s in range(NST)
            ]

            def start_norm(h, s, a_ps):
                # reciprocal of a [1,512] row runs on one DVE lane (~3.3us);
                # kick it off now, finish the broadcast+scale much later so
                # the PE-side rep matmul never waits on it.
                rc = rcpool.tile([65, 512], F32R, tag="rc")
                with nc.allow_low_precision("f32r is wire-identical to f32"):
                    nc.vector.reciprocal(rc[64:65, :], a_ps[64:65, :])
                return (h, s, a_ps, rc)

            def finish_norm(h, s, a_ps, rc):
                qsl = slice(s * 512, (s + 1) * 512)
                r_ps = rps.tile([64, 512], F32, tag="rep")
                nc.tensor.matmul(
                    r_ps[:], ones65[64:65, :], rc[64:65, :], start=True, stop=True
                )
                an = rcpool.tile([64, 512], F32, tag="an")
                nc.vector.tensor_copy(an[:], a_ps[0:64, :])
                nc.vector.tensor_tensor(attnT[h][:, qsl], an[:], r_ps[:], ALU.mult)

            def emit_exp(h, s, kts, bs, pe_t):
                if kts[-1] < 8 * s and _BIG_EXP:
                    nc.scalar.activation(
                        pe_t[:, 0 : len(kts), :],
                        bs[:, 0 : len(kts), :],
                        AF.Exp,
                        scale=0.125,
                    )
                    return
                for j, kt in enumerate(kts):
                    if kt < 8 * s:
                        nc.scalar.activation(
                            pe_t[:, j, :], bs[:, j, :], AF.Exp, scale=0.125
                        )
                        continue
                    ktp = kt - 8 * s
                    cgm = ktp // 2
                    if cgm > 0:
                        nc.vector.tensor_copy(
                            pe_t[:, j, 0 : cgm * 128], zeros384[:, 0 : cgm * 128]
                        )
                    nc.scalar.activation(
                        pe_t[:, j, cgm * 128 :],
                        bs[:, j, cgm * 128 :],
                        AF.Exp,
                        scale=0.125,
                    )
                    nc.vector.tensor_tensor(
                        pe_t[:, j, cgm * 128 : (cgm + 1) * 128],
                        pe_t[:, j, cgm * 128 : (cgm + 1) * 128],
                        tm_sb[:, ktp, :],
                        ALU.mult,
                    )

            # pipeline state
            pend_pv = []    # (h, s, a_ps, pe_t, kts, nkt)
            pend_norm = []  # (due_batch, norm_args)
            batch_no = [0]

            def flush_pv(keep):
                while len(pend_pv) > keep:
                    h, s, a_ps, pe_t, kts, nkt = pend_pv.pop(0)
                    for j, kt in enumerate(kts):
                        nc.tensor.matmul(
                            a_ps[:],
                            vaug[h][:, kt, 0:65],
                            pe_t[:, j, :],
                            start=(kt == 0),
                            stop=(kt == nkt - 1),
                        )
                    if kts[-1] == nkt - 1:
                        pend_norm.append((batch_no[0] + 4, start_norm(h, s, a_ps)))

            def flush_norms(force=False):
                while pend_norm and (force or pend_norm[0][0] <= batch_no[0]):
                    _, args = pend_norm.pop(0)
                    finish_norm(*args)

            for h, s in units:
                nkt = 8 * s + 8
                # backstop: a_ps slots recycle every 2 units, so any norm
                # still pending must be emitted before this unit's alloc
                flush_norms(force=True)
                a_ps = apsp.tile([65, 512], F32, tag="attn")
                qsl = slice(s * 512, (s + 1) * 512)
                for kt0 in range(0, nkt, BK):
                    kts = list(range(kt0, min(kt0 + BK, nkt)))
                    bs = sps.tile([128, BK, 512], F32, tag="s")
                    for j, kt in enumerate(kts):
                        nc.tensor.matmul(
                            bs[:, j, :],
                            s_lhsT(h, slice(kt * 128, (kt + 1) * 128)),
                            s_rhs(h, qsl),
                            start=True,
                            stop=True,
                        )
                    batch_no[0] += 1
                    flush_pv(LAG)
                    flush_norms()
                    pe_t = pepool.tile([128, BK, 512], F32R, tag="pe")
                    emit_exp(h, s, kts, bs, pe_t)
                    pend_pv.append((h, s, a_ps, pe_t, kts, nkt))
            flush_pv(0)
            flush_norms(force=True)

        # --- phase D: partial output projection ---
        with (
            tc.tile_pool(name="ob", bufs=3) as opool,
            tc.tile_pool(name="d_ps", bufs=2, space="PSUM") as dps,
        ):
            for tj in range(NQT if _STOP_AFTER == "full" else 0):
                tsl = slice(tj * 128, (tj + 1) * 128)
                p1 = dps.tile([128, 512], F32, tag="o1")
                p2 = dps.tile([128, 256], F32, tag="o2")
                lhs = tuple(attnT[h][:, tsl] for h in range(HPG))
                rhs2 = tuple(wo_sb[h][:, :] for h in range(HPG))
                for h in range(HPG):
                    nc.tensor.matmul(
                        p1[:], lhs[h], rhs2[h][:, 0:512],
                        start=(h == 0), stop=(h == HPG - 1),
                    )
                for h in range(HPG):
                    nc.tensor.matmul(
                        p2[:], lhs[h], rhs2[h][:, 512:768],
                        start=(h == 0), stop=(h == HPG - 1),
                    )
                ob = opool.tile([128, C], F32, tag="ob")
                nc.vector.tensor_copy(ob[:, 0:512], p1[:])
                nc.vector.tensor_copy(ob[:, 512:768], p2[:])
                nc.sync.dma_start(out[tsl, :], ob[:])

    nc.compile()
    return nc


def _get_nc():
    if "nc" not in _CACHE:
        _CACHE["nc"] = build_nc()
    return _CACHE["nc"]


def make_in_maps(inputs):
    """Shard full inputs into 8 per-core input maps."""
    x = np.ascontiguousarray(np.asarray(inputs["x"], dtype=np.float32)).reshape(T, C)
    W_qkv = np.asarray(inputs["W_qkv"], dtype=np.float32)
    b_qkv = np.asarray(inputs["b_qkv"], dtype=np.float32)
    W_out = np.asarray(inputs["W_out"], dtype=np.float32)

    diag = (np.arange(128)[None, :] >= np.arange(128)[:, None]).astype(np.float32)
    ones = np.ones((128, 128), np.float32)
    zeros = np.zeros((128, 128), np.float32)
    tmask = {}
    for qh in (0, 1):
        m = np.empty((128, 8, 128), np.float32)
        for ktp in range(8):
            if qh == 0:
                m[:, ktp] = diag if ktp % 2 == 0 else zeros
            else:
                m[:, ktp] = ones if ktp % 2 == 0 else diag
        tmask[qh] = m

    xr = x.reshape(NTT, 128, C)
    in_maps = []
    for c in range(N_CORES):
        g, qh = c // 2, c % 2
        sl = slice(g * GCH, (g + 1) * GCH)
        in_maps.append(
            {
                "x": x,
                "xq": np.ascontiguousarray(xr[qh::2].reshape(TQ, C)),
                "wq": np.ascontiguousarray(W_qkv[:, 0 * C + g * GCH : 0 * C + (g + 1) * GCH]),
                "wk": np.ascontiguousarray(W_qkv[:, 1 * C + g * GCH : 1 * C + (g + 1) * GCH]),
                "wv": np.ascontiguousarray(W_qkv[:, 2 * C + g * GCH : 2 * C + (g + 1) * GCH]),
                "bq": np.ascontiguousarray(b_qkv[0 * C + g * GCH : 0 * C + (g + 1) * GCH]),
                "bk": np.ascontiguousarray(b_qkv[1 * C + g * GCH : 1 * C + (g + 1) * GCH]),
                "bv": np.ascontiguousarray(b_qkv[2 * C + g * GCH : 2 * C + (g + 1) * GCH]),
                "wo": np.ascontiguousarray(W_out[sl, :]),
                "tmask": tmask[qh],
            }
        )
    return in_maps


def combine_outputs(parts, b_out):
    """Sum head-group partials per parity, reassemble rows, add bias."""
    out = np.zeros((T, C), np.float32)
    orow = out.reshape(NTT, 128, C)
    for qh in (0, 1):
        acc = parts[qh].astype(np.float32).copy()
        for g in range(1, 4):
            acc += parts[2 * g + qh]
        orow[qh::2] = acc.reshape(NQT, 128, C)
    out += np.asarray(b_out, dtype=np.float32)[None, :]
    return out.reshape(1, T, C)


def _run(inputs, trace=False, tmpdir=None):
    nc = _get_nc()
    in_maps = make_in_maps(inputs)
    res = bass_utils.run_bass_kernel_spmd(
        nc, in_maps, core_ids=list(range(N_CORES)), trace=trace, tmpdir=tmpdir
    )
    parts = [np.asarray(res.results[c]["out"]) for c in range(N_CORES)]
    return combine_outputs(parts, inputs["b_out"]), res


def kernel(**inputs):
    out, _ = _run(inputs)
    return out
